# revision 22
# baseline (speedup 1.0000x reference)
"""AttentionPool2d (masked, 100-mask sparse attention) on 8 TRN2 NeuronCores.

Algorithm notes
---------------
The reference returns out[0] - only the cls/mean query token. So per (b, h)
we only need scores0[m] = q0 . k[m], the 100-mask softmax over keys, the sum
over masks, and one weighted sum over v. Per-core sharding is by head:
core c owns heads {2c, 2c+1} = E-channels [128c, 128c+128). q/k/v weight
rows and c_w columns are sharded accordingly (weights fully partitioned,
no replication); x / pos_emb / (subsampled) mask are replicated.

Perf design (final):
- all large inputs are host-packed to bf16 (halves HBM traffic; rel-err
  budget is 2e-2, measured ~4e-3).
- sigmoid(x) == 0.5*tanh(x/2)+0.5 so every ACT op (tanh, exp) lives in the
  single `exp_and_others` table set; a dummy exp preloads it once.
- DMA issue split across sync HWDGE + gpsimd queues, ordered by first need
  (x0/pos first, cwt/cbt last); x is split in halves so the XS assembly and
  mean-reduce start on the first half while the second streams in.
- the two heads of each batch share one softmax chain (paired [NM, 2*LP]
  mask/exp tiles, per-head row sums via a 3D DVE reduce), and PE program
  order interleaves the two chains with the later projection groups so
  chain latency hides under projection work. Do NOT write two matmul
  accumulation groups into column ranges of one PSUM tile - it compiles
  and passes CoreSim but the NEFF dies at runtime; separate tiles per
  group (pool-padded to a full bank) are required.
- PSUM: 4 single-bank accumulators (K0 | K1 | V0+q0b0 | V1+q0b1) issued as
  contiguous groups (interleaved groups on one tile deadlock the tile
  scheduler); s/w/c-proj tiles rotate through a 4-buffer PSUM pool.
- final cross-core reduction: AllGather of the per-core c-proj partial
  [128, 16] (o-major) + local tree-sum + bias (AG mesh ~5us vs AR ~18us).
- output is returned o-major [128, 2*8] and unpacked to [B, O] on host.

The token axis is padded 197 -> 198 per block (pad cols are zero in x/pos
so K/V pad cols are bias-only; mask pad col is zeroed and the exp row-sum
gets a "-1" correction; the w-pad col is excluded from the v-reduction).
"""
import os

import numpy as np

B = 2
H = 16
E = 1024
SP = 14
S = SP * SP          # 196
NM = 100
L = S + 1            # 197
LP = L + 1           # 198 padded
HD = 64
NET = 8              # e-tiles of 128
NCORES = 8
SCALE = HD ** -0.5   # 0.125
HALF = 4 * LP        # x half-width

_STATE = {}


def _build():
    import concourse.bass as bass
    import concourse.mybir as mybir
    from concourse import bacc, tile

    F32 = mybir.dt.float32
    BF16 = mybir.dt.bfloat16
    AF = mybir.ActivationFunctionType
    ALU = mybir.AluOpType

    nc = bacc.Bacc("TRN2", target_bir_lowering=False, debug=False,
                   num_devices=NCORES)

    x_ap = nc.dram_tensor("x", [B, 128, NET * LP], BF16, kind="ExternalInput").ap()
    pos_ap = nc.dram_tensor("pos_t", [128, NET * LP], BF16, kind="ExternalInput").ap()
    kw_ap = nc.dram_tensor("kw", [128, NET * 128], BF16, kind="ExternalInput").ap()
    vw_ap = nc.dram_tensor("vw", [128, NET * 128], BF16, kind="ExternalInput").ap()
    qw_ap = nc.dram_tensor("qw", [128, NET * 128], BF16, kind="ExternalInput").ap()
    qkvb_ap = nc.dram_tensor("qkvb", [128, 3], F32, kind="ExternalInput").ap()
    cwt_ap = nc.dram_tensor("cwt", [128, E], BF16, kind="ExternalInput").ap()
    cbt_ap = nc.dram_tensor("cbt", [128, 2 * NET], F32, kind="ExternalInput").ap()
    mask_ap = nc.dram_tensor("mask", [NM, B * S], BF16, kind="ExternalInput").ap()
    out_ap = nc.dram_tensor("out", [128, 2 * NET], F32, kind="ExternalOutput").ap()

    with tile.TileContext(nc) as tc:
        with (
            tc.tile_pool(name="sb", bufs=1) as sb,
            tc.tile_pool(name="sb4", bufs=4) as sb4,
            tc.tile_pool(name="ps_kv", bufs=1, space="PSUM") as ps_kv,
            tc.tile_pool(name="ps_sw", bufs=4, space="PSUM") as ps_sw,
            tc.tile_pool(name="dram", bufs=1, space="DRAM") as dram,
        ):
            # ---- input DMAs, strictly ordered by first need ----
            # sync HWDGE: x0h0, pos, x0h1, kw, qw, x1 halves; gpsimd: mask,
            # vw, cwt, qkvb, cbt (late-need). Queues drain packet-round-
            # robin through the shared SDMA engines.
            X = [sb.tile([128, NET * LP], BF16, tag=f"x{b}", name=f"x{b}")
                 for b in range(B)]
            PT = sb.tile([128, NET * LP], BF16, tag="pt")
            nc.sync.dma_start(X[0][:, 0:HALF], x_ap[0, :, 0:HALF])
            nc.sync.dma_start(PT[:, 0:HALF], pos_ap[:, 0:HALF])
            nc.sync.dma_start(X[0][:, HALF:2 * HALF], x_ap[0, :, HALF:2 * HALF])
            nc.sync.dma_start(PT[:, HALF:2 * HALF], pos_ap[:, HALF:2 * HALF])
            KW = sb.tile([128, NET * 128], BF16, tag="kw")
            nc.sync.dma_start(KW[:], kw_ap[:])
            QW = sb.tile([128, NET * 128], BF16, tag="qw")
            nc.sync.dma_start(QW[:], qw_ap[:])
            for h in range(2):
                nc.sync.dma_start(X[1][:, h * HALF:(h + 1) * HALF],
                                  x_ap[1, :, h * HALF:(h + 1) * HALF])
            MIN = sb.tile([NM, B * S], BF16, tag="min")
            nc.gpsimd.dma_start(MIN[:], mask_ap[:])
            VW = sb.tile([128, NET * 128], BF16, tag="vw")
            nc.gpsimd.dma_start(VW[:], vw_ap[:])
            CWT = sb.tile([128, E], BF16, tag="cwt")
            nc.gpsimd.dma_start(CWT[:], cwt_ap[:])
            QKVB = sb.tile([128, 3], F32, tag="qkvb")
            nc.gpsimd.dma_start(QKVB[:], qkvb_ap[:])
            CBT = sb.tile([128, 2 * NET], F32, tag="cbt")
            nc.gpsimd.dma_start(CBT[:], cbt_ap[:])


            # ---- ACT table preload (exp_and_others: exp + tanh) ----
            dummy = sb.tile([1, 2], F32, tag="dummy")
            nc.vector.memset(dummy[:], 0.0)
            nc.scalar.activation(dummy[:], dummy[:], AF.Exp)

            # ---- constants ----
            ones_q = sb.tile([128, NM], F32, tag="ones_q")
            nc.vector.memset(ones_q[:], 1.0)
            ones_r = sb.tile([NM, HD], F32, tag="ones_r")
            nc.vector.memset(ones_r[:], 1.0)

            # ---- XS assembly (per half): XS = x + pos; col0 = mean+pos0 ----
            XS = [sb.tile([128, NET * LP], BF16, tag=f"xs{b}", name=f"xs{b}")
                  for b in range(B)]
            for b in range(B):
                for h in range(2):
                    sl = slice(h * HALF, (h + 1) * HALF)
                    nc.vector.tensor_add(XS[b][:, sl], X[b][:, sl], PT[:, sl])
                    msum = sb.tile([128, 4], F32, tag=f"msum{b}{h}")
                    nc.vector.reduce_sum(
                        msum[:],
                        X[b][:, sl].rearrange("p (n c) -> p n c", c=LP),
                        axis=mybir.AxisListType.X)
                    msd = sb.tile([128, 4], BF16, tag=f"msd{b}{h}")
                    nc.vector.tensor_scalar_mul(msd[:], msum[:], 1.0 / S)
                    nc.vector.tensor_add(XS[b][:, h * HALF:(h + 1) * HALF:LP],
                                         msd[:], PT[:, h * HALF:(h + 1) * HALF:LP])

            # ---- PSUM accumulators: 4 single banks ----
            bankK0 = ps_kv.tile([128, LP], F32, tag="bankK0", name="bankK0")
            bankK1 = ps_kv.tile([128, LP], F32, tag="bankK1", name="bankK1")
            bankV0 = ps_kv.tile([128, LP + 2], F32, tag="bankV0", name="bankV0")
            bankV1 = ps_kv.tile([128, LP + 2], F32, tag="bankV1", name="bankV1")
            K_ps = [bankK0, bankK1]
            V_ps = [bankV0[:, 0:LP], bankV1[:, 0:LP]]
            q0_ps = [bankV0[:, LP:LP + 2], bankV1[:, LP:LP + 2]]

            def group(out, wt, b, q0=False):
                for et in range(NET):
                    rhs = (XS[b][:, et * LP:(et + 1) * LP] if not q0 else
                           XS[b][:, et * LP:(et + 1) * LP:LP - 1])
                    nc.tensor.matmul(out, wt[:, et * 128:(et + 1) * 128], rhs,
                                     start=(et == 0), stop=(et == NET - 1))

            # PE: K0 with both q0 groups interleaved per e-tile (three
            # different PSUM banks, so the interleave is legal); pulls the
            # q0 path ~2.5us earlier so scores are not gated on it.
            for et in range(NET):
                nc.tensor.matmul(K_ps[0][:],
                                 KW[:, et * 128:(et + 1) * 128],
                                 XS[0][:, et * LP:(et + 1) * LP],
                                 start=(et == 0), stop=(et == NET - 1))
                for b in range(B):
                    nc.tensor.matmul(q0_ps[b],
                                     QW[:, et * 128:(et + 1) * 128],
                                     XS[b][:, et * LP:(et + 1) * LP:LP - 1],
                                     start=(et == 0), stop=(et == NET - 1))

            # ---- masks: sigmoid via 0.5*tanh(x/2)+0.5 (same set as exp) ----
            mt = sb.tile([NM, B * S], F32, tag="mt")
            nc.scalar.activation(mt[:], MIN[:], AF.Tanh, scale=0.5)
            # M2[b] = the b-mask duplicated for both heads: [NM, 2*LP]
            M2 = [sb.tile([NM, 2 * LP], F32, tag=f"m2_{b}", name=f"m2_{b}")
                  for b in range(B)]
            for b in range(B):
                for h in range(2):
                    nc.vector.tensor_scalar(
                        M2[b][:, h * LP + 1:h * LP + L],
                        mt[:, b * S:(b + 1) * S], 0.5, 0.5,
                        ALU.mult, ALU.add)
                nc.vector.memset(M2[b][:, 0:2 * LP:LP], 1.0)
                nc.vector.memset(M2[b][:, L:2 * LP:LP], 0.0)



            # DVE: K bias b0, q0 scale/bias, Q0R broadcast
            K_sb = [sb.tile([128, LP], BF16, tag=f"k_sb{b}", name=f"k_sb{b}")
                    for b in range(B)]
            V_sb = [sb.tile([128, LP], F32, tag=f"v_sb{b}", name=f"v_sb{b}")
                    for b in range(B)]
            nc.vector.tensor_scalar_add(K_sb[0][:], K_ps[0][:], QKVB[:, 0:1])
            q0v = sb.tile([128, B], F32, tag="q0v")
            for b in range(B):
                nc.vector.tensor_scalar(q0v[:, b:b + 1], q0_ps[b][:, 0:1],
                                        QKVB[:, 2:3], SCALE, ALU.add, ALU.mult)
            Q0R = []
            for b in range(B):
                q0r = sb.tile([128, NM], BF16, tag=f"q0r{b}")
                for h in range(2):
                    sl = slice(h * HD, (h + 1) * HD)
                    nc.vector.tensor_scalar_mul(q0r[sl, :], ones_q[sl, :],
                                                q0v[sl, b:b + 1])
                Q0R.append(q0r)

            # ---- paired-head softmax chains: one chain per batch ----
            # s_ps[b] = [NM, 2*LP]: head h in cols [h*LP, (h+1)*LP) (two
            # sequential PSUM groups); per-head row sums via a 3D DVE
            # reduce; w matmuls land in the upper/lower partition halves of
            # one [128, LP] tile so a single t_mul/reduce covers both heads.
            S_ps, E_sb, W_ps = {}, {}, {}

            def scores(b):
                for h in range(2):
                    sl = slice(h * HD, (h + 1) * HD)
                    s_ps = ps_sw.tile([NM, LP], F32, tag="sw",
                                      name=f"s{b}{h}", padded_shape=[128, 512])
                    nc.tensor.matmul(s_ps[:], Q0R[b][sl, :], K_sb[b][sl, :],
                                     start=True, stop=True)
                    S_ps[b, h] = s_ps

            def soft(b):
                sm = sb4.tile([NM, 2 * LP], F32, tag="sm", name=f"sm{b}")
                for h in range(2):
                    nc.vector.tensor_mul(sm[:, h * LP:(h + 1) * LP],
                                         S_ps[b, h][:],
                                         M2[b][:, h * LP:(h + 1) * LP])
                e_sb = sb4.tile([NM, 2 * LP], BF16, tag="e", name=f"e{b}")
                nc.scalar.activation(e_sb[:], sm[:], AF.Exp)
                rs = sb4.tile([NM, 2], F32, tag="rs", name=f"rs{b}")
                nc.vector.reduce_sum(
                    rs[:], e_sb[:].rearrange("n (h l) -> n h l", h=2),
                    axis=mybir.AxisListType.X)
                rcol = sb4.tile([NM, 2], F32, tag="rc", name=f"rc{b}")
                nc.vector.tensor_scalar_add(rcol[:], rs[:], -1.0)
                nc.vector.reciprocal(rcol[:], rcol[:])
                rrep = sb4.tile([NM, 2 * HD], BF16, tag="rrep", name=f"rr{b}")
                for h in range(2):
                    nc.vector.tensor_scalar_mul(
                        rrep[:, h * HD:(h + 1) * HD], ones_r[:],
                        rcol[:, h:h + 1])
                E_sb[b] = (e_sb, rrep)

            def wsum(b):
                e_sb, rrep = E_sb[b]
                for h in range(2):
                    w_ps = ps_sw.tile([HD, LP], F32, tag="sw",
                                      name=f"w{b}{h}", padded_shape=[128, 512])
                    nc.tensor.matmul(w_ps[:], rrep[:, h * HD:(h + 1) * HD],
                                     e_sb[:, h * LP:(h + 1) * LP],
                                     start=True, stop=True)
                    W_ps[b, h] = w_ps

            def attn(b):
                # w pad col = sum_n r_n != 0 and V pad col = vb: exclude the
                # pad col from the weighted-v reduction.
                for h in range(2):
                    sl = slice(h * HD, (h + 1) * HD)
                    t_mul = sb4.tile([HD, L], F32, tag="t_mul",
                                     name=f"t{b}{h}")
                    nc.vector.tensor_mul(t_mul[:], W_ps[b, h][:, 0:L],
                                         V_sb[b][sl, 0:L])
                    nc.vector.reduce_sum(A0[sl, b:b + 1], t_mul[:],
                                         axis=mybir.AxisListType.X)

            A0 = sb.tile([128, B], F32, tag="a0")

            scores(0)
            soft(0)
            group(K_ps[1][:], KW, 1)                      # PE: K1
            nc.vector.tensor_scalar_add(K_sb[1][:], K_ps[1][:], QKVB[:, 0:1])
            scores(1)
            soft(1)
            wsum(0)
            group(V_ps[0], VW, 0)                         # PE: V0
            nc.vector.tensor_scalar_add(V_sb[0][:], V_ps[0], QKVB[:, 1:2])
            wsum(1)
            attn(0)
            group(V_ps[1], VW, 1)                         # PE: V1
            nc.vector.tensor_scalar_add(V_sb[1][:], V_ps[1], QKVB[:, 1:2])
            attn(1)

            # ---- c-proj transposed: part[p, 2*ot+b] = sum_ch A0 * c_w ----
            A0r = sb.tile([128, B], BF16, tag="a0r")
            nc.vector.tensor_scalar_add(A0r[:], A0[:], 0.0)
            o_ps = ps_sw.tile([128, 2 * NET], F32, tag="sw", name="o_ps",
                              padded_shape=[128, 512])
            for j in range(NET):
                nc.tensor.matmul(o_ps[:, 2 * j: 2 * j + 2],
                                 CWT[:, j * 128:(j + 1) * 128], A0r[:],
                                 start=True, stop=True)
            part_sb = sb.tile([128, 2 * NET], BF16, tag="part_sb")
            nc.vector.tensor_copy(part_sb[:], o_ps[:])

            # ---- AllGather partials + local tree-sum + bias ----
            part = dram.tile([128, 2 * NET], BF16)
            nc.sync.dma_start(part[:], part_sb[:])
            red = dram.tile([NCORES * 128, 2 * NET], BF16)
            nc.gpsimd.collective_compute(
                "AllGather", mybir.AluOpType.bypass,
                replica_groups=[list(range(NCORES))],
                ins=[part.opt()], outs=[red.opt()])
            G = 2 * NET
            red_sb = sb.tile([128, NCORES * G], BF16, tag="red_sb")
            nc.sync.dma_start(
                red_sb[:].rearrange("p (r c) -> p r c", r=NCORES),
                red[:].rearrange("(r p) c -> p r c", p=128))
            t4 = sb.tile([128, 4 * G], BF16, tag="t4")
            for j in range(4):
                nc.vector.tensor_add(t4[:, j * G:(j + 1) * G],
                                     red_sb[:, 2 * j * G:(2 * j + 1) * G],
                                     red_sb[:, (2 * j + 1) * G:(2 * j + 2) * G])
            t2 = sb.tile([128, 2 * G], BF16, tag="t2")
            for j in range(2):
                nc.vector.tensor_add(t2[:, j * G:(j + 1) * G],
                                     t4[:, 2 * j * G:(2 * j + 1) * G],
                                     t4[:, (2 * j + 1) * G:(2 * j + 2) * G])
            t1 = sb.tile([128, G], F32, tag="t1")
            nc.vector.tensor_add(t1[:], t2[:, 0:G], t2[:, G:2 * G])
            out_sb = sb.tile([128, G], F32, tag="out_sb")
            nc.vector.tensor_add(out_sb[:], t1[:], CBT[:])
            nc.sync.dma_start(out_ap[:], out_sb[:])

    nc.compile()
    return nc


def _get_nc():
    if "nc" not in _STATE:
        _STATE["nc"] = _build()
    return _STATE["nc"]


def _pack_blocks(a, block_in, pad_to, col_off):
    """[rows=8*128, cols=block_in] -> bf16 [128, 8*pad_to], zero elsewhere.

    Block et occupies cols [et*pad_to + col_off, et*pad_to + col_off + block_in).
    """
    a = np.asarray(a, dtype=np.float32)
    t = a.reshape(NET, 128, block_in).transpose(1, 0, 2)  # [128, 8, block_in]
    out = np.zeros((128, NET, pad_to), np.float32)
    out[:, :, col_off:col_off + block_in] = t
    return np.ascontiguousarray(
        out.reshape(128, NET * pad_to)).astype(_bf16())


def _bf16():
    import ml_dtypes
    return ml_dtypes.bfloat16


def host_inputs(inputs):
    x = np.asarray(inputs["x"], np.float32)
    mask_feature = np.asarray(inputs["mask_feature"], np.float32)
    pos_emb = np.asarray(inputs["pos_emb"], np.float32)
    q_w = np.asarray(inputs["q_w"], np.float32)
    q_b = np.asarray(inputs["q_b"], np.float32)
    k_w = np.asarray(inputs["k_w"], np.float32)
    k_b = np.asarray(inputs["k_b"], np.float32)
    v_w = np.asarray(inputs["v_w"], np.float32)
    v_b = np.asarray(inputs["v_b"], np.float32)
    c_w = np.asarray(inputs["c_w"], np.float32)
    c_b = np.asarray(inputs["c_b"], np.float32)

    # replicated tensors (packed layouts, pure data movement + dtype cast)
    x_flat = x.reshape(B, E, S)
    # x block: [0 | x tokens (196) | 0]; pos block: [pos0..pos196 | 0]
    x_packed = np.stack([_pack_blocks(x_flat[b], S, LP, 1) for b in range(B)])
    pos_packed = _pack_blocks(np.ascontiguousarray(pos_emb.T), L, LP, 0)
    # mask[n, b*S + s] = mask_feature[b, n, ::8, ::8]
    mask12 = np.ascontiguousarray(
        mask_feature[:, :, ::8, ::8].reshape(B, NM, S).transpose(1, 0, 2)
        .reshape(NM, B * S)).astype(_bf16())
    # cbt[p, 2*ot+b] = c_b[ot*128+p]
    cbt = np.ascontiguousarray(
        np.repeat(c_b.reshape(NET, 128).T[:, :, None], B, axis=2
                  ).reshape(128, NET * B))

    in_maps = []
    for c in range(NCORES):
        ch = slice(c * 128, (c + 1) * 128)
        in_maps.append({
            "x": x_packed,
            "pos_t": pos_packed,
            "kw": _pack_blocks(np.ascontiguousarray(k_w[ch].T), 128, 128, 0),
            "vw": _pack_blocks(np.ascontiguousarray(v_w[ch].T), 128, 128, 0),
            "qw": _pack_blocks(np.ascontiguousarray(q_w[ch].T), 128, 128, 0),
            "qkvb": np.ascontiguousarray(
                np.stack([k_b[ch], v_b[ch], q_b[ch]], axis=1)),
            "cwt": np.ascontiguousarray(c_w[:, ch].T).astype(_bf16()),
            "cbt": cbt,
            "mask": mask12,
        })
    return in_maps


def unpack_out(out):
    # out[p, 2*ot+b] -> [B, O]
    o = np.asarray(out, np.float32).reshape(128, NET, B)
    return np.ascontiguousarray(o.transpose(2, 1, 0).reshape(B, E))


def kernel(**inputs):
    in_maps = host_inputs(inputs)

    from concourse.bass_utils import run_bass_kernel_spmd

    nc = _get_nc()
    trace = bool(int(os.environ.get("KERNEL_TRACE", "0")))
    if trace:
        try:
            import ntff_hook
            ntff_hook.install()
        except Exception:
            pass
    res = run_bass_kernel_spmd(nc, in_maps, list(range(NCORES)), trace=trace)
    _STATE["last_exec_ns"] = res.exec_time_ns
    _STATE["last_results"] = res
    return unpack_out(res.results[0]["out"])


# revision 23
# speedup vs baseline: 1.1588x; 1.1588x over previous
"""AttentionPool2d (masked, 100-mask sparse attention) on 8 TRN2 NeuronCores.

Algorithm notes
---------------
The reference returns out[0] - only the cls/mean query token. So per (b, h)
we only need scores0[m] = q0 . k[m], the 100-mask softmax over keys, the sum
over masks, and one weighted sum over v. Per-core sharding is by head:
core c owns heads {2c, 2c+1} = E-channels [128c, 128c+128). q/k/v weight
rows and c_w columns are sharded accordingly (weights fully partitioned,
no replication); x / pos_emb / (subsampled) mask are replicated.

Perf design (final):
- all large inputs are host-packed to bf16 (halves HBM traffic; rel-err
  budget is 2e-2, measured ~4e-3).
- sigmoid(x) == 0.5*tanh(x/2)+0.5 so every ACT op (tanh, exp) lives in the
  single `exp_and_others` table set; a dummy exp preloads it once.
- DMA issue split across sync HWDGE + gpsimd queues, ordered by first need
  (x0/pos first, cwt/cbt last); x is split in halves so the XS assembly and
  mean-reduce start on the first half while the second streams in.
- the two heads of each batch share one softmax chain (paired [NM, 2*LP]
  mask/exp tiles, per-head row sums via a 3D DVE reduce), and PE program
  order interleaves the two chains with the later projection groups so
  chain latency hides under projection work. Do NOT write two matmul
  accumulation groups into column ranges of one PSUM tile - it compiles
  and passes CoreSim but the NEFF dies at runtime; separate tiles per
  group (pool-padded to a full bank) are required.
- PSUM: 4 single-bank accumulators (K0 | K1 | V0+q0b0 | V1+q0b1) issued as
  contiguous groups (interleaved groups on one tile deadlock the tile
  scheduler); s/w/c-proj tiles rotate through a 4-buffer PSUM pool.
- final cross-core reduction: AllGather of the per-core c-proj partial
  [128, 16] (o-major) + local tree-sum + bias (AG mesh ~5us vs AR ~18us).
- output is returned o-major [128, 2*8] and unpacked to [B, O] on host.

The token axis is padded 197 -> 198 per block (pad cols are zero in x/pos
so K/V pad cols are bias-only; mask pad col is zeroed and the exp row-sum
gets a "-1" correction; the w-pad col is excluded from the v-reduction).
"""
import os

import numpy as np

B = 2
H = 16
E = 1024
SP = 14
S = SP * SP          # 196
NM = 100
L = S + 1            # 197
LP = L + 1           # 198 padded
HD = 64
NET = 8              # e-tiles of 128
NCORES = 8
SCALE = HD ** -0.5   # 0.125
HALF = 4 * LP        # x half-width

_STATE = {}


def _build():
    import concourse.bass as bass
    import concourse.mybir as mybir
    from concourse import bacc, tile

    F32 = mybir.dt.float32
    BF16 = mybir.dt.bfloat16
    AF = mybir.ActivationFunctionType
    ALU = mybir.AluOpType

    nc = bacc.Bacc("TRN2", target_bir_lowering=False, debug=False,
                   num_devices=NCORES)

    x_ap = nc.dram_tensor("x", [B, 128, NET * LP], BF16, kind="ExternalInput").ap()
    pos_ap = nc.dram_tensor("pos_t", [128, NET * LP], BF16, kind="ExternalInput").ap()
    kw_ap = nc.dram_tensor("kw", [128, NET * 128], BF16, kind="ExternalInput").ap()
    vw_ap = nc.dram_tensor("vw", [128, NET * 128], BF16, kind="ExternalInput").ap()
    qw_ap = nc.dram_tensor("qw", [128, NET * 128], BF16, kind="ExternalInput").ap()
    qkvb_ap = nc.dram_tensor("qkvb", [128, 3], F32, kind="ExternalInput").ap()
    cwt_ap = nc.dram_tensor("cwt", [128, E], BF16, kind="ExternalInput").ap()
    cbt_ap = nc.dram_tensor("cbt", [128, 2 * NET], F32, kind="ExternalInput").ap()
    mask_ap = nc.dram_tensor("mask", [NM, B * S], BF16, kind="ExternalInput").ap()
    out_ap = nc.dram_tensor("out", [128, 2 * NET], F32, kind="ExternalOutput").ap()

    with tile.TileContext(nc) as tc:
        with (
            tc.tile_pool(name="sb", bufs=1) as sb,
            tc.tile_pool(name="sb4", bufs=4) as sb4,
            tc.tile_pool(name="ps_kv", bufs=1, space="PSUM") as ps_kv,
            tc.tile_pool(name="ps_sw", bufs=4, space="PSUM") as ps_sw,
            tc.tile_pool(name="dram", bufs=1, space="DRAM") as dram,
        ):
            # ---- input DMAs, strictly ordered by first need ----
            # sync HWDGE: x0h0, pos, x0h1, kw, qw, x1 halves; gpsimd: mask,
            # vw, cwt, qkvb, cbt (late-need). Queues drain packet-round-
            # robin through the shared SDMA engines.
            X = [sb.tile([128, NET * LP], BF16, tag=f"x{b}", name=f"x{b}")
                 for b in range(B)]
            PT = sb.tile([128, NET * LP], BF16, tag="pt")
            nc.sync.dma_start(X[0][:, 0:HALF], x_ap[0, :, 0:HALF])
            nc.sync.dma_start(PT[:, 0:HALF], pos_ap[:, 0:HALF])
            KW = sb.tile([128, NET * 128], BF16, tag="kw")
            nc.sync.dma_start(KW[:], kw_ap[:])
            nc.sync.dma_start(X[0][:, HALF:2 * HALF], x_ap[0, :, HALF:2 * HALF])
            nc.sync.dma_start(PT[:, HALF:2 * HALF], pos_ap[:, HALF:2 * HALF])
            QW = sb.tile([128, NET * 128], BF16, tag="qw")
            nc.sync.dma_start(QW[:], qw_ap[:])
            for h in range(2):
                nc.sync.dma_start(X[1][:, h * HALF:(h + 1) * HALF],
                                  x_ap[1, :, h * HALF:(h + 1) * HALF])
            MIN = sb.tile([NM, B * S], BF16, tag="min")
            nc.gpsimd.dma_start(MIN[:], mask_ap[:])
            VW = sb.tile([128, NET * 128], BF16, tag="vw")
            nc.gpsimd.dma_start(VW[:], vw_ap[:])
            CWT = sb.tile([128, E], BF16, tag="cwt")
            nc.gpsimd.dma_start(CWT[:], cwt_ap[:])
            QKVB = sb.tile([128, 3], F32, tag="qkvb")
            nc.gpsimd.dma_start(QKVB[:], qkvb_ap[:])
            CBT = sb.tile([128, 2 * NET], F32, tag="cbt")
            nc.gpsimd.dma_start(CBT[:], cbt_ap[:])


            # ---- ACT table preload (exp_and_others: exp + tanh) ----
            dummy = sb.tile([1, 2], F32, tag="dummy")
            nc.vector.memset(dummy[:], 0.0)
            nc.scalar.activation(dummy[:], dummy[:], AF.Exp)

            # ---- constants ----
            ones_q = sb.tile([128, NM], F32, tag="ones_q")
            nc.vector.memset(ones_q[:], 1.0)
            ones_r = sb.tile([NM, HD], F32, tag="ones_r")
            nc.vector.memset(ones_r[:], 1.0)

            # ---- XS assembly (per half): XS = x + pos; col0 = mean+pos0 ----
            XS = [sb.tile([128, NET * LP], BF16, tag=f"xs{b}", name=f"xs{b}")
                  for b in range(B)]
            for b in range(B):
                for h in range(2):
                    sl = slice(h * HALF, (h + 1) * HALF)
                    nc.vector.tensor_add(XS[b][:, sl], X[b][:, sl], PT[:, sl])
                    msum = sb.tile([128, 4], F32, tag=f"msum{b}{h}")
                    nc.vector.reduce_sum(
                        msum[:],
                        X[b][:, sl].rearrange("p (n c) -> p n c", c=LP),
                        axis=mybir.AxisListType.X)
                    msd = sb.tile([128, 4], BF16, tag=f"msd{b}{h}")
                    nc.vector.tensor_scalar_mul(msd[:], msum[:], 1.0 / S)
                    nc.vector.tensor_add(XS[b][:, h * HALF:(h + 1) * HALF:LP],
                                         msd[:], PT[:, h * HALF:(h + 1) * HALF:LP])

            # ---- PSUM accumulators: 4 single banks ----
            bankK0 = ps_kv.tile([128, LP], F32, tag="bankK0", name="bankK0")
            bankK1 = ps_kv.tile([128, LP], F32, tag="bankK1", name="bankK1")
            bankV0 = ps_kv.tile([128, LP + 2], F32, tag="bankV0", name="bankV0")
            bankV1 = ps_kv.tile([128, LP + 2], F32, tag="bankV1", name="bankV1")
            K_ps = [bankK0, bankK1]
            V_ps = [bankV0[:, 0:LP], bankV1[:, 0:LP]]
            q0_ps = [bankV0[:, LP:LP + 2], bankV1[:, LP:LP + 2]]

            def group(out, wt, b, q0=False):
                for et in range(NET):
                    rhs = (XS[b][:, et * LP:(et + 1) * LP] if not q0 else
                           XS[b][:, et * LP:(et + 1) * LP:LP - 1])
                    nc.tensor.matmul(out, wt[:, et * 128:(et + 1) * 128], rhs,
                                     start=(et == 0), stop=(et == NET - 1))

            # PE: K0, then q0 b0/b1 interleaved per e-tile (shared QW
            # lhsT; the two q0 groups live in different PSUM banks so the
            # pairwise interleave is legal and weight reloads can be elided)
            group(K_ps[0][:], KW, 0)
            for et in range(NET):
                for b in range(B):
                    nc.tensor.matmul(q0_ps[b],
                                     QW[:, et * 128:(et + 1) * 128],
                                     XS[b][:, et * LP:(et + 1) * LP:LP - 1],
                                     start=(et == 0), stop=(et == NET - 1))

            # ---- masks: sigmoid via 0.5*tanh(x/2)+0.5 (same set as exp) ----
            mt = sb.tile([NM, B * S], F32, tag="mt")
            nc.scalar.activation(mt[:], MIN[:], AF.Tanh, scale=0.5)
            # M2[b] = the b-mask duplicated for both heads: [NM, 2*LP]
            M2 = [sb.tile([NM, 2 * LP], F32, tag=f"m2_{b}", name=f"m2_{b}")
                  for b in range(B)]
            for b in range(B):
                for h in range(2):
                    nc.vector.tensor_scalar(
                        M2[b][:, h * LP + 1:h * LP + L],
                        mt[:, b * S:(b + 1) * S], 0.5, 0.5,
                        ALU.mult, ALU.add)
                nc.vector.memset(M2[b][:, 0:2 * LP:LP], 1.0)
                nc.vector.memset(M2[b][:, L:2 * LP:LP], 0.0)



            # DVE: K bias b0, q0 scale/bias, Q0R broadcast
            K_sb = [sb.tile([128, LP], BF16, tag=f"k_sb{b}", name=f"k_sb{b}")
                    for b in range(B)]
            V_sb = [sb.tile([128, LP], F32, tag=f"v_sb{b}", name=f"v_sb{b}")
                    for b in range(B)]
            nc.vector.tensor_scalar_add(K_sb[0][:], K_ps[0][:], QKVB[:, 0:1])
            q0v = sb.tile([128, B], F32, tag="q0v")
            for b in range(B):
                nc.vector.tensor_scalar(q0v[:, b:b + 1], q0_ps[b][:, 0:1],
                                        QKVB[:, 2:3], SCALE, ALU.add, ALU.mult)
            Q0R = []
            for b in range(B):
                q0r = sb.tile([128, NM], BF16, tag=f"q0r{b}")
                for h in range(2):
                    sl = slice(h * HD, (h + 1) * HD)
                    nc.vector.tensor_scalar_mul(q0r[sl, :], ones_q[sl, :],
                                                q0v[sl, b:b + 1])
                Q0R.append(q0r)

            # ---- paired-head softmax chains: one chain per batch ----
            # s_ps[b] = [NM, 2*LP]: head h in cols [h*LP, (h+1)*LP) (two
            # sequential PSUM groups); per-head row sums via a 3D DVE
            # reduce; w matmuls land in the upper/lower partition halves of
            # one [128, LP] tile so a single t_mul/reduce covers both heads.
            S_ps, E_sb, W_ps = {}, {}, {}

            def scores(b):
                for h in range(2):
                    sl = slice(h * HD, (h + 1) * HD)
                    s_ps = ps_sw.tile([NM, LP], F32, tag="sw",
                                      name=f"s{b}{h}", padded_shape=[128, 512])
                    nc.tensor.matmul(s_ps[:], Q0R[b][sl, :], K_sb[b][sl, :],
                                     start=True, stop=True)
                    S_ps[b, h] = s_ps

            def soft(b):
                sm = sb4.tile([NM, 2 * LP], F32, tag="sm", name=f"sm{b}")
                for h in range(2):
                    nc.vector.tensor_mul(sm[:, h * LP:(h + 1) * LP],
                                         S_ps[b, h][:],
                                         M2[b][:, h * LP:(h + 1) * LP])
                e_sb = sb4.tile([NM, 2 * LP], BF16, tag="e", name=f"e{b}")
                nc.scalar.activation(e_sb[:], sm[:], AF.Exp)
                rs = sb4.tile([NM, 2], F32, tag="rs", name=f"rs{b}")
                nc.vector.reduce_sum(
                    rs[:], e_sb[:].rearrange("n (h l) -> n h l", h=2),
                    axis=mybir.AxisListType.X)
                rcol = sb4.tile([NM, 2], F32, tag="rc", name=f"rc{b}")
                nc.vector.tensor_scalar_add(rcol[:], rs[:], -1.0)
                nc.vector.reciprocal(rcol[:], rcol[:])
                rrep = sb4.tile([NM, 2 * HD], BF16, tag="rrep", name=f"rr{b}")
                for h in range(2):
                    nc.vector.tensor_scalar_mul(
                        rrep[:, h * HD:(h + 1) * HD], ones_r[:],
                        rcol[:, h:h + 1])
                E_sb[b] = (e_sb, rrep)

            def wsum(b):
                e_sb, rrep = E_sb[b]
                for h in range(2):
                    w_ps = ps_sw.tile([HD, LP], F32, tag="sw",
                                      name=f"w{b}{h}", padded_shape=[128, 512])
                    nc.tensor.matmul(w_ps[:], rrep[:, h * HD:(h + 1) * HD],
                                     e_sb[:, h * LP:(h + 1) * LP],
                                     start=True, stop=True)
                    W_ps[b, h] = w_ps

            def attn(b):
                # w pad col = sum_n r_n != 0 and V pad col = vb: exclude the
                # pad col from the weighted-v reduction.
                for h in range(2):
                    sl = slice(h * HD, (h + 1) * HD)
                    t_mul = sb4.tile([HD, L], F32, tag="t_mul",
                                     name=f"t{b}{h}")
                    nc.vector.tensor_mul(t_mul[:], W_ps[b, h][:, 0:L],
                                         V_sb[b][sl, 0:L])
                    nc.vector.reduce_sum(A0[sl, b:b + 1], t_mul[:],
                                         axis=mybir.AxisListType.X)

            A0 = sb.tile([128, B], F32, tag="a0")

            scores(0)
            soft(0)
            group(K_ps[1][:], KW, 1)                      # PE: K1
            nc.vector.tensor_scalar_add(K_sb[1][:], K_ps[1][:], QKVB[:, 0:1])
            scores(1)
            soft(1)
            wsum(0)
            group(V_ps[0], VW, 0)                         # PE: V0
            nc.vector.tensor_scalar_add(V_sb[0][:], V_ps[0], QKVB[:, 1:2])
            wsum(1)
            attn(0)
            group(V_ps[1], VW, 1)                         # PE: V1
            nc.vector.tensor_scalar_add(V_sb[1][:], V_ps[1], QKVB[:, 1:2])
            attn(1)

            # ---- c-proj transposed: part[p, 2*ot+b] = sum_ch A0 * c_w ----
            A0r = sb.tile([128, B], BF16, tag="a0r")
            nc.vector.tensor_scalar_add(A0r[:], A0[:], 0.0)
            o_ps = ps_sw.tile([128, 2 * NET], F32, tag="sw", name="o_ps",
                              padded_shape=[128, 512])
            for j in range(NET):
                nc.tensor.matmul(o_ps[:, 2 * j: 2 * j + 2],
                                 CWT[:, j * 128:(j + 1) * 128], A0r[:],
                                 start=True, stop=True)
            part_sb = sb.tile([128, 2 * NET], BF16, tag="part_sb")
            nc.vector.tensor_copy(part_sb[:], o_ps[:])

            # ---- AllGather partials + local tree-sum + bias ----
            part = dram.tile([128, 2 * NET], BF16)
            nc.sync.dma_start(part[:], part_sb[:])
            red = dram.tile([NCORES * 128, 2 * NET], BF16)
            nc.gpsimd.collective_compute(
                "AllGather", mybir.AluOpType.bypass,
                replica_groups=[list(range(NCORES))],
                ins=[part.opt()], outs=[red.opt()])
            G = 2 * NET
            red_sb = sb.tile([128, NCORES * G], BF16, tag="red_sb")
            nc.sync.dma_start(
                red_sb[:].rearrange("p (r c) -> p r c", r=NCORES),
                red[:].rearrange("(r p) c -> p r c", p=128))
            t4 = sb.tile([128, 4 * G], BF16, tag="t4")
            for j in range(4):
                nc.vector.tensor_add(t4[:, j * G:(j + 1) * G],
                                     red_sb[:, 2 * j * G:(2 * j + 1) * G],
                                     red_sb[:, (2 * j + 1) * G:(2 * j + 2) * G])
            t2 = sb.tile([128, 2 * G], BF16, tag="t2")
            for j in range(2):
                nc.vector.tensor_add(t2[:, j * G:(j + 1) * G],
                                     t4[:, 2 * j * G:(2 * j + 1) * G],
                                     t4[:, (2 * j + 1) * G:(2 * j + 2) * G])
            t1 = sb.tile([128, G], F32, tag="t1")
            nc.vector.tensor_add(t1[:], t2[:, 0:G], t2[:, G:2 * G])
            out_sb = sb.tile([128, G], F32, tag="out_sb")
            nc.vector.tensor_add(out_sb[:], t1[:], CBT[:])
            nc.sync.dma_start(out_ap[:], out_sb[:])

    nc.compile()
    return nc


def _get_nc():
    if "nc" not in _STATE:
        _STATE["nc"] = _build()
    return _STATE["nc"]


def _pack_blocks(a, block_in, pad_to, col_off):
    """[rows=8*128, cols=block_in] -> bf16 [128, 8*pad_to], zero elsewhere.

    Block et occupies cols [et*pad_to + col_off, et*pad_to + col_off + block_in).
    """
    a = np.asarray(a, dtype=np.float32)
    t = a.reshape(NET, 128, block_in).transpose(1, 0, 2)  # [128, 8, block_in]
    out = np.zeros((128, NET, pad_to), np.float32)
    out[:, :, col_off:col_off + block_in] = t
    return np.ascontiguousarray(
        out.reshape(128, NET * pad_to)).astype(_bf16())


def _bf16():
    import ml_dtypes
    return ml_dtypes.bfloat16


def host_inputs(inputs):
    x = np.asarray(inputs["x"], np.float32)
    mask_feature = np.asarray(inputs["mask_feature"], np.float32)
    pos_emb = np.asarray(inputs["pos_emb"], np.float32)
    q_w = np.asarray(inputs["q_w"], np.float32)
    q_b = np.asarray(inputs["q_b"], np.float32)
    k_w = np.asarray(inputs["k_w"], np.float32)
    k_b = np.asarray(inputs["k_b"], np.float32)
    v_w = np.asarray(inputs["v_w"], np.float32)
    v_b = np.asarray(inputs["v_b"], np.float32)
    c_w = np.asarray(inputs["c_w"], np.float32)
    c_b = np.asarray(inputs["c_b"], np.float32)

    # replicated tensors (packed layouts, pure data movement + dtype cast)
    x_flat = x.reshape(B, E, S)
    # x block: [0 | x tokens (196) | 0]; pos block: [pos0..pos196 | 0]
    x_packed = np.stack([_pack_blocks(x_flat[b], S, LP, 1) for b in range(B)])
    pos_packed = _pack_blocks(np.ascontiguousarray(pos_emb.T), L, LP, 0)
    # mask[n, b*S + s] = mask_feature[b, n, ::8, ::8]
    mask12 = np.ascontiguousarray(
        mask_feature[:, :, ::8, ::8].reshape(B, NM, S).transpose(1, 0, 2)
        .reshape(NM, B * S)).astype(_bf16())
    # cbt[p, 2*ot+b] = c_b[ot*128+p]
    cbt = np.ascontiguousarray(
        np.repeat(c_b.reshape(NET, 128).T[:, :, None], B, axis=2
                  ).reshape(128, NET * B))

    in_maps = []
    for c in range(NCORES):
        ch = slice(c * 128, (c + 1) * 128)
        in_maps.append({
            "x": x_packed,
            "pos_t": pos_packed,
            "kw": _pack_blocks(np.ascontiguousarray(k_w[ch].T), 128, 128, 0),
            "vw": _pack_blocks(np.ascontiguousarray(v_w[ch].T), 128, 128, 0),
            "qw": _pack_blocks(np.ascontiguousarray(q_w[ch].T), 128, 128, 0),
            "qkvb": np.ascontiguousarray(
                np.stack([k_b[ch], v_b[ch], q_b[ch]], axis=1)),
            "cwt": np.ascontiguousarray(c_w[:, ch].T).astype(_bf16()),
            "cbt": cbt,
            "mask": mask12,
        })
    return in_maps


def unpack_out(out):
    # out[p, 2*ot+b] -> [B, O]
    o = np.asarray(out, np.float32).reshape(128, NET, B)
    return np.ascontiguousarray(o.transpose(2, 1, 0).reshape(B, E))


def kernel(**inputs):
    in_maps = host_inputs(inputs)

    from concourse.bass_utils import run_bass_kernel_spmd

    nc = _get_nc()
    trace = bool(int(os.environ.get("KERNEL_TRACE", "0")))
    if trace:
        try:
            import ntff_hook
            ntff_hook.install()
        except Exception:
            pass
    res = run_bass_kernel_spmd(nc, in_maps, list(range(NCORES)), trace=trace)
    _STATE["last_exec_ns"] = res.exec_time_ns
    _STATE["last_results"] = res
    return unpack_out(res.results[0]["out"])


# revision 24
# speedup vs baseline: 1.3028x; 1.1243x over previous
"""AttentionPool2d (masked, 100-mask sparse attention) on 8 TRN2 NeuronCores.

Algorithm notes
---------------
The reference returns out[0] - only the cls/mean query token. So per (b, h)
we only need scores0[m] = q0 . k[m], the 100-mask softmax over keys, the sum
over masks, and one weighted sum over v. Per-core sharding is by head:
core c owns heads {2c, 2c+1} = E-channels [128c, 128c+128). q/k/v weight
rows and c_w columns are sharded accordingly (weights fully partitioned,
no replication); x / pos_emb / (subsampled) mask are replicated.

Perf design (final):
- all large inputs are host-packed to bf16 (halves HBM traffic; rel-err
  budget is 2e-2, measured ~4e-3).
- sigmoid(x) == 0.5*tanh(x/2)+0.5 so every ACT op (tanh, exp) lives in the
  single `exp_and_others` table set; a dummy exp preloads it once.
- DMA issue split across sync HWDGE + gpsimd queues, ordered by first need
  (x0/pos first, cwt/cbt last); x is split in halves so the XS assembly and
  mean-reduce start on the first half while the second streams in.
- the two heads of each batch share one softmax chain (paired [NM, 2*LP]
  mask/exp tiles, per-head row sums via a 3D DVE reduce), and PE program
  order interleaves the two chains with the later projection groups so
  chain latency hides under projection work. Do NOT write two matmul
  accumulation groups into column ranges of one PSUM tile - it compiles
  and passes CoreSim but the NEFF dies at runtime; separate tiles per
  group (pool-padded to a full bank) are required.
- PSUM: 4 single-bank accumulators (K0 | K1 | V0+q0b0 | V1+q0b1) issued as
  contiguous groups (interleaved groups on one tile deadlock the tile
  scheduler); s/w/c-proj tiles rotate through a 4-buffer PSUM pool.
- final cross-core reduction: AllGather of the per-core c-proj partial
  [128, 16] (o-major) + local tree-sum + bias (AG mesh ~5us vs AR ~18us).
- output is returned o-major [128, 2*8] and unpacked to [B, O] on host.

The token axis is padded 197 -> 198 per block (pad cols are zero in x/pos
so K/V pad cols are bias-only; mask pad col is zeroed and the exp row-sum
gets a "-1" correction; the w-pad col is excluded from the v-reduction).
"""
import os

import numpy as np

B = 2
H = 16
E = 1024
SP = 14
S = SP * SP          # 196
NM = 100
L = S + 1            # 197
LP = L + 1           # 198 padded
HD = 64
NET = 8              # e-tiles of 128
NCORES = 8
SCALE = HD ** -0.5   # 0.125
HALF = 4 * LP        # x half-width

_STATE = {}


def _build():
    import concourse.bass as bass
    import concourse.mybir as mybir
    from concourse import bacc, tile

    F32 = mybir.dt.float32
    BF16 = mybir.dt.bfloat16
    AF = mybir.ActivationFunctionType
    ALU = mybir.AluOpType

    nc = bacc.Bacc("TRN2", target_bir_lowering=False, debug=False,
                   num_devices=NCORES)

    x_ap = nc.dram_tensor("x", [B, 128, NET * LP], BF16, kind="ExternalInput").ap()
    pos_ap = nc.dram_tensor("pos_t", [128, NET * LP], BF16, kind="ExternalInput").ap()
    kw_ap = nc.dram_tensor("kw", [128, NET * 128], BF16, kind="ExternalInput").ap()
    vw_ap = nc.dram_tensor("vw", [128, NET * 128], BF16, kind="ExternalInput").ap()
    qw_ap = nc.dram_tensor("qw", [128, NET * 128], BF16, kind="ExternalInput").ap()
    qkvb_ap = nc.dram_tensor("qkvb", [128, 3], F32, kind="ExternalInput").ap()
    cwt_ap = nc.dram_tensor("cwt", [128, E], BF16, kind="ExternalInput").ap()
    cbt_ap = nc.dram_tensor("cbt", [128, 2 * NET], F32, kind="ExternalInput").ap()
    mask_ap = nc.dram_tensor("mask", [NM, B * S], BF16, kind="ExternalInput").ap()
    out_ap = nc.dram_tensor("out", [128, 2 * NET], F32, kind="ExternalOutput").ap()

    with tile.TileContext(nc) as tc:
        with (
            tc.tile_pool(name="sb", bufs=1) as sb,
            tc.tile_pool(name="sb4", bufs=4) as sb4,
            tc.tile_pool(name="ps_kv", bufs=1, space="PSUM") as ps_kv,
            tc.tile_pool(name="ps_sw", bufs=4, space="PSUM") as ps_sw,
            tc.tile_pool(name="dram", bufs=1, space="DRAM") as dram,
        ):
            # ---- input DMAs, strictly ordered by first need ----
            # sync HWDGE: x0h0, pos, x0h1, kw, qw, x1 halves; gpsimd: mask,
            # vw, cwt, qkvb, cbt (late-need). Queues drain packet-round-
            # robin through the shared SDMA engines.
            X = [sb.tile([128, NET * LP], BF16, tag=f"x{b}", name=f"x{b}")
                 for b in range(B)]
            PT = sb.tile([128, NET * LP], BF16, tag="pt")
            nc.sync.dma_start(X[0][:, 0:HALF], x_ap[0, :, 0:HALF])
            nc.sync.dma_start(PT[:, 0:HALF], pos_ap[:, 0:HALF])
            KW = sb.tile([128, NET * 128], BF16, tag="kw")
            nc.sync.dma_start(KW[:], kw_ap[:])
            nc.sync.dma_start(X[0][:, HALF:2 * HALF], x_ap[0, :, HALF:2 * HALF])
            nc.sync.dma_start(PT[:, HALF:2 * HALF], pos_ap[:, HALF:2 * HALF])
            QW = sb.tile([128, NET * 128], BF16, tag="qw")
            nc.sync.dma_start(QW[:], qw_ap[:])
            for h in range(2):
                nc.sync.dma_start(X[1][:, h * HALF:(h + 1) * HALF],
                                  x_ap[1, :, h * HALF:(h + 1) * HALF])
            MIN = sb.tile([NM, B * S], BF16, tag="min")
            nc.gpsimd.dma_start(MIN[:], mask_ap[:])
            VW = sb.tile([128, NET * 128], BF16, tag="vw")
            nc.gpsimd.dma_start(VW[:], vw_ap[:])
            CWT = sb.tile([128, E], BF16, tag="cwt")
            nc.gpsimd.dma_start(CWT[:], cwt_ap[:])
            QKVB = sb.tile([128, 3], F32, tag="qkvb")
            nc.gpsimd.dma_start(QKVB[:], qkvb_ap[:])
            CBT = sb.tile([128, 2 * NET], F32, tag="cbt")
            nc.gpsimd.dma_start(CBT[:], cbt_ap[:])


            # ---- ACT table preload (exp_and_others: exp + tanh) ----
            dummy = sb.tile([1, 2], F32, tag="dummy")
            nc.vector.memset(dummy[:], 0.0)
            nc.scalar.activation(dummy[:], dummy[:], AF.Exp)

            # ---- constants ----
            ones_q = sb.tile([128, NM], F32, tag="ones_q")
            nc.vector.memset(ones_q[:], 1.0)
            ones_r = sb.tile([NM, HD], F32, tag="ones_r")
            nc.vector.memset(ones_r[:], 1.0)

            # ---- masks: sigmoid via 0.5*tanh(x/2)+0.5 (same set as exp) ----
            mt = sb.tile([NM, B * S], F32, tag="mt")
            nc.scalar.activation(mt[:], MIN[:], AF.Tanh, scale=0.5)
            # M2[b] = the b-mask duplicated for both heads: [NM, 2*LP]
            M2 = [sb.tile([NM, 2 * LP], F32, tag=f"m2_{b}", name=f"m2_{b}")
                  for b in range(B)]
            for b in range(B):
                for h in range(2):
                    nc.vector.tensor_scalar(
                        M2[b][:, h * LP + 1:h * LP + L],
                        mt[:, b * S:(b + 1) * S], 0.5, 0.5,
                        ALU.mult, ALU.add)
                nc.vector.memset(M2[b][:, 0:2 * LP:LP], 1.0)
                nc.vector.memset(M2[b][:, L:2 * LP:LP], 0.0)

            # ---- XS assembly (per half): XS = x + pos; col0 = mean+pos0 ----
            XS = [sb.tile([128, NET * LP], BF16, tag=f"xs{b}", name=f"xs{b}")
                  for b in range(B)]
            for b in range(B):
                for h in range(2):
                    sl = slice(h * HALF, (h + 1) * HALF)
                    nc.vector.tensor_add(XS[b][:, sl], X[b][:, sl], PT[:, sl])
                    msum = sb.tile([128, 4], F32, tag=f"msum{b}{h}")
                    nc.vector.reduce_sum(
                        msum[:],
                        X[b][:, sl].rearrange("p (n c) -> p n c", c=LP),
                        axis=mybir.AxisListType.X)
                    msd = sb.tile([128, 4], BF16, tag=f"msd{b}{h}")
                    nc.vector.tensor_scalar_mul(msd[:], msum[:], 1.0 / S)
                    nc.vector.tensor_add(XS[b][:, h * HALF:(h + 1) * HALF:LP],
                                         msd[:], PT[:, h * HALF:(h + 1) * HALF:LP])

            # ---- PSUM accumulators: 4 single banks ----
            bankK0 = ps_kv.tile([128, LP], F32, tag="bankK0", name="bankK0")
            bankK1 = ps_kv.tile([128, LP], F32, tag="bankK1", name="bankK1")
            bankV0 = ps_kv.tile([128, LP + 2], F32, tag="bankV0", name="bankV0")
            bankV1 = ps_kv.tile([128, LP + 2], F32, tag="bankV1", name="bankV1")
            K_ps = [bankK0, bankK1]
            V_ps = [bankV0[:, 0:LP], bankV1[:, 0:LP]]
            q0_ps = [bankV0[:, LP:LP + 2], bankV1[:, LP:LP + 2]]

            def group(out, wt, b, q0=False):
                for et in range(NET):
                    rhs = (XS[b][:, et * LP:(et + 1) * LP] if not q0 else
                           XS[b][:, et * LP:(et + 1) * LP:LP - 1])
                    nc.tensor.matmul(out, wt[:, et * 128:(et + 1) * 128], rhs,
                                     start=(et == 0), stop=(et == NET - 1))

            # PE: K0, then q0 b0/b1 interleaved per e-tile (shared QW
            # lhsT; the two q0 groups live in different PSUM banks so the
            # pairwise interleave is legal and weight reloads can be elided)
            group(K_ps[0][:], KW, 0)
            for et in range(NET):
                for b in range(B):
                    nc.tensor.matmul(q0_ps[b],
                                     QW[:, et * 128:(et + 1) * 128],
                                     XS[b][:, et * LP:(et + 1) * LP:LP - 1],
                                     start=(et == 0), stop=(et == NET - 1))



            # DVE: K bias b0, q0 scale/bias, Q0R broadcast
            K_sb = [sb.tile([128, LP], BF16, tag=f"k_sb{b}", name=f"k_sb{b}")
                    for b in range(B)]
            V_sb = [sb.tile([128, LP], F32, tag=f"v_sb{b}", name=f"v_sb{b}")
                    for b in range(B)]
            nc.vector.tensor_scalar_add(K_sb[0][:], K_ps[0][:], QKVB[:, 0:1])
            q0v = sb.tile([128, B], F32, tag="q0v")
            for b in range(B):
                nc.vector.tensor_scalar(q0v[:, b:b + 1], q0_ps[b][:, 0:1],
                                        QKVB[:, 2:3], SCALE, ALU.add, ALU.mult)
            Q0R = []
            for b in range(B):
                q0r = sb.tile([128, NM], BF16, tag=f"q0r{b}")
                for h in range(2):
                    sl = slice(h * HD, (h + 1) * HD)
                    nc.vector.tensor_scalar_mul(q0r[sl, :], ones_q[sl, :],
                                                q0v[sl, b:b + 1])
                Q0R.append(q0r)

            # ---- paired-head softmax chains: one chain per batch ----
            # s_ps[b] = [NM, 2*LP]: head h in cols [h*LP, (h+1)*LP) (two
            # sequential PSUM groups); per-head row sums via a 3D DVE
            # reduce; w matmuls land in the upper/lower partition halves of
            # one [128, LP] tile so a single t_mul/reduce covers both heads.
            S_ps, E_sb, W_ps = {}, {}, {}

            def scores(b):
                for h in range(2):
                    sl = slice(h * HD, (h + 1) * HD)
                    s_ps = ps_sw.tile([NM, LP], F32, tag="sw",
                                      name=f"s{b}{h}", padded_shape=[128, 512])
                    nc.tensor.matmul(s_ps[:], Q0R[b][sl, :], K_sb[b][sl, :],
                                     start=True, stop=True)
                    S_ps[b, h] = s_ps

            def soft(b):
                sm = sb4.tile([NM, 2 * LP], F32, tag="sm", name=f"sm{b}")
                for h in range(2):
                    nc.vector.tensor_mul(sm[:, h * LP:(h + 1) * LP],
                                         S_ps[b, h][:],
                                         M2[b][:, h * LP:(h + 1) * LP])
                e_sb = sb4.tile([NM, 2 * LP], BF16, tag="e", name=f"e{b}")
                nc.scalar.activation(e_sb[:], sm[:], AF.Exp)
                rs = sb4.tile([NM, 2], F32, tag="rs", name=f"rs{b}")
                nc.vector.reduce_sum(
                    rs[:], e_sb[:].rearrange("n (h l) -> n h l", h=2),
                    axis=mybir.AxisListType.X)
                rcol = sb4.tile([NM, 2], F32, tag="rc", name=f"rc{b}")
                nc.vector.tensor_scalar_add(rcol[:], rs[:], -1.0)
                nc.vector.reciprocal(rcol[:], rcol[:])
                rrep = sb4.tile([NM, 2 * HD], BF16, tag="rrep", name=f"rr{b}")
                for h in range(2):
                    nc.vector.tensor_scalar_mul(
                        rrep[:, h * HD:(h + 1) * HD], ones_r[:],
                        rcol[:, h:h + 1])
                E_sb[b] = (e_sb, rrep)

            def wsum(b):
                e_sb, rrep = E_sb[b]
                for h in range(2):
                    w_ps = ps_sw.tile([HD, LP], F32, tag="sw",
                                      name=f"w{b}{h}", padded_shape=[128, 512])
                    nc.tensor.matmul(w_ps[:], rrep[:, h * HD:(h + 1) * HD],
                                     e_sb[:, h * LP:(h + 1) * LP],
                                     start=True, stop=True)
                    W_ps[b, h] = w_ps

            def attn(b):
                # w pad col = sum_n r_n != 0 and V pad col = vb: exclude the
                # pad col from the weighted-v reduction.
                for h in range(2):
                    sl = slice(h * HD, (h + 1) * HD)
                    t_mul = sb4.tile([HD, L], F32, tag="t_mul",
                                     name=f"t{b}{h}")
                    nc.vector.tensor_mul(t_mul[:], W_ps[b, h][:, 0:L],
                                         V_sb[b][sl, 0:L])
                    nc.vector.reduce_sum(A0[sl, b:b + 1], t_mul[:],
                                         axis=mybir.AxisListType.X)

            A0 = sb.tile([128, B], F32, tag="a0")

            scores(0)
            soft(0)
            group(K_ps[1][:], KW, 1)                      # PE: K1
            nc.vector.tensor_scalar_add(K_sb[1][:], K_ps[1][:], QKVB[:, 0:1])
            scores(1)
            soft(1)
            wsum(0)
            group(V_ps[0], VW, 0)                         # PE: V0
            nc.vector.tensor_scalar_add(V_sb[0][:], V_ps[0], QKVB[:, 1:2])
            wsum(1)
            attn(0)
            group(V_ps[1], VW, 1)                         # PE: V1
            nc.vector.tensor_scalar_add(V_sb[1][:], V_ps[1], QKVB[:, 1:2])
            attn(1)

            # ---- c-proj transposed: part[p, 2*ot+b] = sum_ch A0 * c_w ----
            A0r = sb.tile([128, B], BF16, tag="a0r")
            nc.vector.tensor_scalar_add(A0r[:], A0[:], 0.0)
            o_ps = ps_sw.tile([128, 2 * NET], F32, tag="sw", name="o_ps",
                              padded_shape=[128, 512])
            for j in range(NET):
                nc.tensor.matmul(o_ps[:, 2 * j: 2 * j + 2],
                                 CWT[:, j * 128:(j + 1) * 128], A0r[:],
                                 start=True, stop=True)
            part_sb = sb.tile([128, 2 * NET], BF16, tag="part_sb")
            nc.vector.tensor_copy(part_sb[:], o_ps[:])

            # ---- AllGather partials + local tree-sum + bias ----
            part = dram.tile([128, 2 * NET], BF16)
            nc.sync.dma_start(part[:], part_sb[:])
            red = dram.tile([NCORES * 128, 2 * NET], BF16)
            nc.gpsimd.collective_compute(
                "AllGather", mybir.AluOpType.bypass,
                replica_groups=[list(range(NCORES))],
                ins=[part.opt()], outs=[red.opt()])
            G = 2 * NET
            red_sb = sb.tile([128, NCORES * G], BF16, tag="red_sb")
            nc.sync.dma_start(
                red_sb[:].rearrange("p (r c) -> p r c", r=NCORES),
                red[:].rearrange("(r p) c -> p r c", p=128))
            t4 = sb.tile([128, 4 * G], BF16, tag="t4")
            for j in range(4):
                nc.vector.tensor_add(t4[:, j * G:(j + 1) * G],
                                     red_sb[:, 2 * j * G:(2 * j + 1) * G],
                                     red_sb[:, (2 * j + 1) * G:(2 * j + 2) * G])
            t2 = sb.tile([128, 2 * G], BF16, tag="t2")
            for j in range(2):
                nc.vector.tensor_add(t2[:, j * G:(j + 1) * G],
                                     t4[:, 2 * j * G:(2 * j + 1) * G],
                                     t4[:, (2 * j + 1) * G:(2 * j + 2) * G])
            t1 = sb.tile([128, G], F32, tag="t1")
            nc.vector.tensor_add(t1[:], t2[:, 0:G], t2[:, G:2 * G])
            out_sb = sb.tile([128, G], F32, tag="out_sb")
            nc.vector.tensor_add(out_sb[:], t1[:], CBT[:])
            nc.sync.dma_start(out_ap[:], out_sb[:])

    nc.compile()
    return nc


def _get_nc():
    if "nc" not in _STATE:
        _STATE["nc"] = _build()
    return _STATE["nc"]


def _pack_blocks(a, block_in, pad_to, col_off):
    """[rows=8*128, cols=block_in] -> bf16 [128, 8*pad_to], zero elsewhere.

    Block et occupies cols [et*pad_to + col_off, et*pad_to + col_off + block_in).
    """
    a = np.asarray(a, dtype=np.float32)
    t = a.reshape(NET, 128, block_in).transpose(1, 0, 2)  # [128, 8, block_in]
    out = np.zeros((128, NET, pad_to), np.float32)
    out[:, :, col_off:col_off + block_in] = t
    return np.ascontiguousarray(
        out.reshape(128, NET * pad_to)).astype(_bf16())


def _bf16():
    import ml_dtypes
    return ml_dtypes.bfloat16


def host_inputs(inputs):
    x = np.asarray(inputs["x"], np.float32)
    mask_feature = np.asarray(inputs["mask_feature"], np.float32)
    pos_emb = np.asarray(inputs["pos_emb"], np.float32)
    q_w = np.asarray(inputs["q_w"], np.float32)
    q_b = np.asarray(inputs["q_b"], np.float32)
    k_w = np.asarray(inputs["k_w"], np.float32)
    k_b = np.asarray(inputs["k_b"], np.float32)
    v_w = np.asarray(inputs["v_w"], np.float32)
    v_b = np.asarray(inputs["v_b"], np.float32)
    c_w = np.asarray(inputs["c_w"], np.float32)
    c_b = np.asarray(inputs["c_b"], np.float32)

    # replicated tensors (packed layouts, pure data movement + dtype cast)
    x_flat = x.reshape(B, E, S)
    # x block: [0 | x tokens (196) | 0]; pos block: [pos0..pos196 | 0]
    x_packed = np.stack([_pack_blocks(x_flat[b], S, LP, 1) for b in range(B)])
    pos_packed = _pack_blocks(np.ascontiguousarray(pos_emb.T), L, LP, 0)
    # mask[n, b*S + s] = mask_feature[b, n, ::8, ::8]
    mask12 = np.ascontiguousarray(
        mask_feature[:, :, ::8, ::8].reshape(B, NM, S).transpose(1, 0, 2)
        .reshape(NM, B * S)).astype(_bf16())
    # cbt[p, 2*ot+b] = c_b[ot*128+p]
    cbt = np.ascontiguousarray(
        np.repeat(c_b.reshape(NET, 128).T[:, :, None], B, axis=2
                  ).reshape(128, NET * B))

    in_maps = []
    for c in range(NCORES):
        ch = slice(c * 128, (c + 1) * 128)
        in_maps.append({
            "x": x_packed,
            "pos_t": pos_packed,
            "kw": _pack_blocks(np.ascontiguousarray(k_w[ch].T), 128, 128, 0),
            "vw": _pack_blocks(np.ascontiguousarray(v_w[ch].T), 128, 128, 0),
            "qw": _pack_blocks(np.ascontiguousarray(q_w[ch].T), 128, 128, 0),
            "qkvb": np.ascontiguousarray(
                np.stack([k_b[ch], v_b[ch], q_b[ch]], axis=1)),
            "cwt": np.ascontiguousarray(c_w[:, ch].T).astype(_bf16()),
            "cbt": cbt,
            "mask": mask12,
        })
    return in_maps


def unpack_out(out):
    # out[p, 2*ot+b] -> [B, O]
    o = np.asarray(out, np.float32).reshape(128, NET, B)
    return np.ascontiguousarray(o.transpose(2, 1, 0).reshape(B, E))


def kernel(**inputs):
    in_maps = host_inputs(inputs)

    from concourse.bass_utils import run_bass_kernel_spmd

    nc = _get_nc()
    trace = bool(int(os.environ.get("KERNEL_TRACE", "0")))
    if trace:
        try:
            import ntff_hook
            ntff_hook.install()
        except Exception:
            pass
    res = run_bass_kernel_spmd(nc, in_maps, list(range(NCORES)), trace=trace)
    _STATE["last_exec_ns"] = res.exec_time_ns
    _STATE["last_results"] = res
    return unpack_out(res.results[0]["out"])
